# revision 1
# baseline (speedup 1.0000x reference)
"""Trainium2 Bass kernel for nn_Classifier (segment_reduce).

Computation (reference semantics):
  attn  = concat(emb, pos) @ W_attn + b_attn          (S, T, 1)
  w     = softmax(attn, axis=1)                        per-segment over T
  segv  = sum_t w * emb                                (S, BERT)
  vecs  = segment_sum(segv, segment_ids, 64)           (64, BERT)
  out   = sigmoid(lrelu(lrelu(vecs@W1+b1)@W2+b2)@W3+b3)

Sharding: data-parallel over S across 8 NeuronCores (32 segments each),
AllReduce of the comment partials (bf16), replicated MLP.

Structure:
 - b_attn shifts all logits of a segment equally -> softmax-invariant -> dropped.
 - softmax normalization 1/den is folded into the host-built one-hot
   segment->comment matrix, scaled on device by 1/den per segment.
 - exp() without max-subtraction: logits ~ N(0, 0.6^2), safe in fp32.
 - Per-segment pipeline: DMA(ep fp32) ->
   4x DVE fused mul+reduce (logits, fp32) -> scalar EXP ->
   8 fp32 pooling matmuls (PSUM row) -> scalar copy -> scatter.
   The PSUM->SBUF copy for segment s is emitted one iteration late so
   the in-order scalar stream never stalls exp(s+1) behind matmuls(s).
 - MLP weights/biases are fed from host as bf16 (half the DMA bytes),
   loaded in ~0.26MB chunks interleaved with the segment loop.
 - Comment partials are built transposed (cmT[feat, com] via matmuls
   with segvecs as stationary), AllReduced in bf16 (98KB), and fed to a
   transpose-free "chained-T" MLP: activations stay feature-major so
   each layer's output is directly the next layer's stationary input.
"""

import sys

sys.path.insert(0, "/opt/trn_rl_repo")

import ml_dtypes
import numpy as np

BF16 = ml_dtypes.bfloat16

# Full-problem dims (hardcoded per contract)
S, T, BERT, POS = 256, 512, 768, 128
FEAT = BERT + POS
H1 = 1024
NCLS = 6
NCOM = 64
NCORES = 8

_CACHE = {}


def build_nc(n_cores, sl, t, bert, pos, h1, ncls, ncom):
    """Build the SPMD Bass program for one core (sl segments/core)."""
    import concourse.bass as bass
    import concourse.mybir as mybir
    import concourse.tile as tile
    from concourse import bacc
    from concourse.masks import make_identity

    f32 = mybir.dt.float32
    bf16 = mybir.dt.bfloat16
    AF = mybir.ActivationFunctionType
    OP = mybir.AluOpType
    AX = mybir.AxisListType

    feat = bert + pos
    nt = t // 128          # token tiles per segment
    nj1 = bert // 128      # k tiles layer1 (6)
    nj2 = h1 // 128        # k tiles layer2/3 (8)
    nh = h1 // 128         # output chunks of h1 (8)

    nc = bacc.Bacc(
        "TRN2", target_bir_lowering=False, debug=False, num_devices=n_cores
    )

    emb_d = nc.dram_tensor("emb", [sl, t, bert], f32, kind="ExternalInput").ap()
    pos_d = nc.dram_tensor("pos", [sl, t, pos], f32, kind="ExternalInput").ap()
    wab_d = nc.dram_tensor("wab", [128, feat], f32, kind="ExternalInput").ap()
    oneh_d = nc.dram_tensor("oneh", [sl, ncom], f32, kind="ExternalInput").ap()
    w1_d = nc.dram_tensor("w1", [bert, h1], bf16, kind="ExternalInput").ap()
    b1_d = nc.dram_tensor("b1", [1, h1], bf16, kind="ExternalInput").ap()
    w2_d = nc.dram_tensor("w2", [h1, h1], bf16, kind="ExternalInput").ap()
    b2_d = nc.dram_tensor("b2", [1, h1], bf16, kind="ExternalInput").ap()
    w3_d = nc.dram_tensor("w3", [h1, ncls], bf16, kind="ExternalInput").ap()
    b3_d = nc.dram_tensor("b3", [1, ncls], bf16, kind="ExternalInput").ap()
    out_d = nc.dram_tensor("out", [ncom, ncls], f32, kind="ExternalOutput").ap()

    with tile.TileContext(nc) as tc:
        with (
            tc.tile_pool(name="const", bufs=1) as const_pool,
            tc.tile_pool(name="ep", bufs=4) as ep_pool,
            tc.tile_pool(name="work", bufs=1) as work,
            tc.tile_pool(name="psv", bufs=2, space="PSUM") as psv,
            tc.tile_pool(name="pmisc", bufs=2, space="PSUM") as pmisc,
            tc.tile_pool(name="dram", bufs=1, space="DRAM") as dram,
        ):
            # ---- constants ----
            wab_sb = const_pool.tile([128, feat], f32)
            nc.sync.dma_start(wab_sb, wab_d)
            oneh_sb = const_pool.tile([sl, ncom], f32)
            nc.sync.dma_start(oneh_sb, oneh_d)
            identity = const_pool.tile([128, 128], f32)
            make_identity(nc, identity)
            ones_sb = const_pool.tile([128, 64], f32)
            nc.gpsimd.memset(ones_sb, 1.0)
            ones_bf = const_pool.tile([128, 64], bf16)
            nc.gpsimd.memset(ones_bf, 1.0)

            # ---- persistent working tiles ----
            L_sb = work.tile([128, nt * sl], f32)     # logits, col = s*nt + i
            E_sb = work.tile([128, nt * sl], f32)     # exp(logits)
            segvecs = work.tile([sl, bert], f32)      # unnormalized segvecs
            dummy = work.tile([128, 1], f32)          # discard for fused reduce

            # ---- MLP weight tiles (bf16 straight from HBM) ----
            w1b = const_pool.tile([128, nj1, h1], bf16)
            w2b = const_pool.tile([128, nj2, h1], bf16)
            w3b = const_pool.tile([128, nj2, ncls], bf16)
            b1b = const_pool.tile([1, h1], bf16)
            b2b = const_pool.tile([1, h1], bf16)
            b3b = const_pool.tile([1, ncls], bf16)

            w1_r = w1_d.rearrange("(j p) h -> p j h", p=128)
            w2_r = w2_d.rearrange("(j p) h -> p j h", p=128)
            w3_r = w3_d.rearrange("(j p) h -> p j h", p=128)

            # One weight-DMA chunk (~0.26MB) or small-tensor load per job;
            # interleaved into the segment loop so the HBM ring stays dense
            # and the loads fully overlap compute.
            dma_jobs = []
            for j in range(nj1):
                dma_jobs.append(
                    lambda j=j: nc.sync.dma_start(w1b[:, j : j + 1, :], w1_r[:, j : j + 1, :])
                )
            for j in range(nj2):
                dma_jobs.append(
                    lambda j=j: nc.sync.dma_start(w2b[:, j : j + 1, :], w2_r[:, j : j + 1, :])
                )

            def _small_loads():
                nc.sync.dma_start(w3b, w3_r)
                nc.sync.dma_start(b1b, b1_d)
                nc.sync.dma_start(b2b, b2_d)
                nc.sync.dma_start(b3b, b3_d)

            dma_jobs.append(_small_loads)

            # ---- main loop over local segments ----
            sv_tiles = {}

            def drain_segvec(sp):
                # PSUM -> SBUF stage (scalar), then partition-scatter DMA.
                stage = work.tile([1, bert], f32, tag="stage", bufs=3, name="stage")
                nc.scalar.copy(stage, sv_tiles.pop(sp))
                nc.sync.dma_start(segvecs[sp : sp + 1, :], stage)

            for s in range(sl):
                ep = ep_pool.tile([128, nt, feat], f32, tag="ep")
                nc.sync.dma_start(
                    ep[:, :, 0:bert],
                    emb_d[s].rearrange("(i p) f -> p i f", p=128),
                )
                nc.sync.dma_start(
                    ep[:, :, bert:feat],
                    pos_d[s].rearrange("(i p) f -> p i f", p=128),
                )
                # weight-load chunk riding the same ring, behind this
                # segment's tiles, ahead of the next segment's.
                if s >= 1 and dma_jobs:
                    dma_jobs.pop(0)()

                # attention logits: fused multiply + free-dim reduce
                for i in range(nt):
                    nc.vector.scalar_tensor_tensor(
                        dummy.broadcast_to([128, feat]),
                        ep[:, i, :],
                        1.0,
                        wab_sb,
                        op0=OP.mult,
                        op1=OP.mult,
                        accum_out=L_sb[:, nt * s + i : nt * s + i + 1],
                    )
                # e = exp(logits), bf16 out for the pooling stationary
                nc.scalar.activation(
                    E_sb[:, nt * s : nt * s + nt],
                    L_sb[:, nt * s : nt * s + nt],
                    AF.Exp,
                )
                # Drain the PREVIOUS segment's pooled row. Emitted here so
                # the scalar stream goes exp(s) -> copy(s-1): copy(s-1)'s
                # wait (on matmuls(s-1)) is already satisfied, so exp(s+1)
                # is never held hostage to matmuls(s).
                if s >= 1:
                    drain_segvec(s - 1)

                # pooling: segvec[s] = E-weighted sum of emb over tokens.
                sv = psv.tile([1, bert], f32, tag="sv")
                sv_tiles[s] = sv
                for i in range(nt):
                    col = nt * s + i
                    for n0 in range(0, bert, 512):
                        n1 = min(n0 + 512, bert)
                        nc.tensor.matmul(
                            sv[0:1, n0:n1],
                            E_sb[:, col : col + 1],
                            ep[:, i, n0:n1],
                            start=(i == 0),
                            stop=(i == nt - 1),
                        )

            drain_segvec(sl - 1)
            while dma_jobs:
                dma_jobs.pop(0)()

            # ---- denominators: den[s] = sum_t e ----
            den_row = pmisc.tile([1, nt * sl], f32, tag="m")
            nc.tensor.matmul(
                den_row, ones_sb[:, 0:1], E_sb, start=True, stop=True
            )
            den_sb = work.tile([1, sl], f32)
            nc.vector.tensor_reduce(
                den_sb,
                den_row.rearrange("p (s i) -> p s i", i=nt),
                axis=AX.X,
                op=OP.add,
            )
            den_col = pmisc.tile([sl, 1], f32, tag="m")
            nc.tensor.matmul(
                den_col, den_sb, ones_sb[0:1, 0:1], start=True, stop=True
            )
            inv_den = work.tile([sl, 1], f32)
            nc.vector.reciprocal(inv_den, den_col)
            oneh_sc = work.tile([sl, ncom], f32)
            nc.vector.tensor_scalar_mul(oneh_sc, oneh_sb, inv_den)

            # ---- comment partials, transposed: cmT[f-chunk, j, c] ----
            cmT = pmisc.tile([128, nj1, ncom], f32, tag="m")
            for j in range(nj1):
                nc.tensor.matmul(
                    cmT[:, j, :],
                    segvecs[:, 128 * j : 128 * (j + 1)],
                    oneh_sc,
                    start=True,
                    stop=True,
                )
            arin_sb = work.tile([128, nj1 * ncom], bf16)
            nc.scalar.copy(arin_sb, cmT.rearrange("p j c -> p (j c)"))

            # ---- AllReduce over cores (bf16 payload, 98KB) ----
            ar_in = dram.tile([128, nj1 * ncom], bf16)
            ar_out = dram.tile([128, nj1 * ncom], bf16)
            nc.sync.dma_start(ar_in, arin_sb)
            nc.gpsimd.collective_compute(
                "AllReduce",
                OP.add,
                replica_groups=[list(range(n_cores))],
                ins=[ar_in.opt()],
                outs=[ar_out.opt()],
            )
            vecsT = work.tile([128, nj1, ncom], bf16)
            nc.sync.dma_start(vecsT.rearrange("p j c -> p (j c)"), ar_out)

            # ---- MLP, feature-major all the way (no transposes) ----
            def linearT(xT, njx, wb, brow, nchunks):
                # hT[p, n, c] = sum_j wb[:, j, 128n+p]^T xT[:, j, c] + b
                hT = pmisc.tile([128, nchunks, ncom], f32, tag="m", name="hT")
                for n in range(nchunks):
                    for j in range(njx):
                        nc.tensor.matmul(
                            hT[:, n, :],
                            wb[:, j, 128 * n : 128 * (n + 1)],
                            xT[:, j, :],
                            start=(j == 0),
                            stop=False,
                        )
                    nc.tensor.matmul(
                        hT[:, n, :],
                        brow[0:1, 128 * n : 128 * (n + 1)],
                        ones_bf[0:1, 0:ncom],
                        start=False,
                        stop=True,
                    )
                yT = work.tile([128, nchunks, ncom], bf16, tag="yT", bufs=2, name="yT")
                nc.scalar.activation(
                    yT.rearrange("p n c -> p (n c)"),
                    hT.rearrange("p n c -> p (n c)"),
                    AF.Lrelu,
                    alpha=0.01,
                )
                return yT

            y1T = linearT(vecsT, nj1, w1b, b1b, nh)
            y2T = linearT(y1T, nj2, w2b, b2b, nh)

            out3 = pmisc.tile([ncls, ncom], f32, tag="m", name="out3")
            for j in range(nj2):
                nc.tensor.matmul(
                    out3,
                    w3b[:, j, :],
                    y2T[:, j, :],
                    start=(j == 0),
                    stop=False,
                )
            nc.tensor.matmul(
                out3,
                b3b[0:1, 0:ncls],
                ones_bf[0:1, 0:ncom],
                start=False,
                stop=True,
            )
            sig_sb = work.tile([ncls, ncom], f32)
            nc.scalar.activation(sig_sb, out3, AF.Sigmoid)
            outT_ps = pmisc.tile([ncom, ncls], f32, tag="m", name="outT")
            nc.tensor.transpose(outT_ps, sig_sb, identity[0:ncls, 0:ncls])
            out_sb = work.tile([ncom, ncls], f32)
            nc.vector.tensor_copy(out_sb, outT_ps)
            nc.sync.dma_start(out_d, out_sb)

    nc.compile()
    return nc


def make_in_maps(
    embeddings,
    position_encodings,
    W_attn,
    W1,
    b1,
    W2,
    b2,
    W3,
    b3,
    segment_ids,
    n_cores,
    ncom,
):
    """Host-side sharding: slice S across cores, build per-core one-hot."""
    f32 = np.float32
    s_total = embeddings.shape[0]
    sl = s_total // n_cores
    feat = embeddings.shape[2] + position_encodings.shape[2]

    wa = np.asarray(W_attn, dtype=f32).reshape(-1)
    wab = np.ascontiguousarray(np.tile(wa[None, :], (128, 1)))
    assert wab.shape == (128, feat)

    seg = np.asarray(segment_ids).astype(np.int64).reshape(-1)
    common = {
        "wab": wab,
        "w1": np.ascontiguousarray(np.asarray(W1, dtype=f32).astype(BF16)),
        "b1": np.ascontiguousarray(np.asarray(b1, dtype=f32).reshape(1, -1).astype(BF16)),
        "w2": np.ascontiguousarray(np.asarray(W2, dtype=f32).astype(BF16)),
        "b2": np.ascontiguousarray(np.asarray(b2, dtype=f32).reshape(1, -1).astype(BF16)),
        "w3": np.ascontiguousarray(np.asarray(W3, dtype=f32).astype(BF16)),
        "b3": np.ascontiguousarray(np.asarray(b3, dtype=f32).reshape(1, -1).astype(BF16)),
    }
    in_maps = []
    for c in range(n_cores):
        oneh = np.zeros((sl, ncom), dtype=f32)
        local = seg[c * sl : (c + 1) * sl]
        oneh[np.arange(sl), local] = 1.0
        in_maps.append(
            {
                "emb": np.ascontiguousarray(
                    embeddings[c * sl : (c + 1) * sl], dtype=f32
                ),
                "pos": np.ascontiguousarray(
                    position_encodings[c * sl : (c + 1) * sl], dtype=f32
                ),
                "oneh": oneh,
                **common,
            }
        )
    return in_maps


def kernel(
    embeddings,
    position_encodings,
    W_attn,
    b_attn,
    W1,
    b1,
    W2,
    b2,
    W3,
    b3,
    segment_ids,
    num_comments,
):
    from concourse.bass_utils import run_bass_kernel_spmd

    assert int(num_comments) == NCOM
    assert embeddings.shape == (S, T, BERT)
    assert position_encodings.shape == (S, T, POS)
    # b_attn shifts every logit of a segment equally -> softmax-invariant.

    key = "full"
    if key not in _CACHE:
        _CACHE[key] = build_nc(NCORES, S // NCORES, T, BERT, POS, H1, NCLS, NCOM)
    nc = _CACHE[key]

    in_maps = make_in_maps(
        embeddings,
        position_encodings,
        W_attn,
        W1,
        b1,
        W2,
        b2,
        W3,
        b3,
        segment_ids,
        NCORES,
        NCOM,
    )
    res = run_bass_kernel_spmd(nc, in_maps, list(range(NCORES)))
    return np.asarray(res.results[0]["out"], dtype=np.float32)



# revision 8
# speedup vs baseline: 1.5957x; 1.5957x over previous
"""Trainium2 Bass kernel for nn_Classifier (segment_reduce).

Computation (reference semantics):
  attn  = concat(emb, pos) @ W_attn + b_attn          (S, T, 1)
  w     = softmax(attn, axis=1)                        per-segment over T
  segv  = sum_t w * emb                                (S, BERT)
  vecs  = segment_sum(segv, segment_ids, 64)           (64, BERT)
  out   = sigmoid(lrelu(lrelu(vecs@W1+b1)@W2+b2)@W3+b3)

Sharding: data-parallel over S across 8 NeuronCores (32 segments each),
AllReduce of the comment partials (bf16), replicated MLP.

Structure (v2, bf16 end-to-end):
 - b_attn shifts all logits of a segment equally -> softmax-invariant -> dropped.
 - Host packs emb+pos per core into bf16, partition-major layout
   [128, sl, nt, 897] with columns [emb 768 | 1.0 | pos 128] so that
   (a) each 4-segment DMA is 128 x 28.7KB contiguous descriptors,
   (b) the pooling matmul over columns [512:769] accumulates the ones
       column -> softmax denominator lands in the pooled row for free.
   W_attn is zero-padded at the ones column so logits are unaffected.
 - Logits: DVE fused mul+reduce (bf16 in, fp32 accum), exp on scalar
   engine (bf16 out) -> pooling matmuls all-bf16 (4x PE stream rate).
 - Softmax 1/den folded into the host-built one-hot segment->comment
   matrix, scaled on device by reciprocal of the free den column.
 - Comment partials built transposed (cmT[feat, com]), AllReduced in
   bf16 (98KB) with a Shared-address output buffer (fast mesh path),
   and fed to a transpose-free feature-major MLP.
 - MLP: bias via k=1 matmuls, LeakyReLU as one DVE scalar_tensor_tensor
   (x*0.01 max x) per layer, final layer emitted directly as [64, 6]
   (y2 chunks stationary), sigmoid as exp(-x) -> 1/(1+e) on DVE so the
   scalar engine never swaps activation tables.
"""

import sys

sys.path.insert(0, "/opt/trn_rl_repo")

import ml_dtypes
import numpy as np

BF16 = ml_dtypes.bfloat16

# Full-problem dims (hardcoded per contract)
S, T, BERT, POS = 256, 512, 768, 128
FEAT = BERT + POS
FEAT2 = FEAT + 1  # [emb 768 | ones 1 | pos 128]
H1 = 1024
NCLS = 6
NCOM = 64
NCORES = 8
SEG_CHUNK = 4  # segments per input DMA

_CACHE = {}


def build_nc(n_cores, sl, t, bert, pos, h1, ncls, ncom):
    """Build the SPMD Bass program for one core (sl segments/core)."""
    import concourse.bass as bass
    import concourse.mybir as mybir
    import concourse.tile as tile
    from concourse import bacc

    f32 = mybir.dt.float32
    bf16 = mybir.dt.bfloat16
    AF = mybir.ActivationFunctionType
    OP = mybir.AluOpType

    feat2 = bert + pos + 1
    nt = t // 128          # token tiles per segment (4)
    nj1 = bert // 128      # k tiles layer1 (6)
    nj2 = h1 // 128        # k tiles layer2/3 (8)
    nh = h1 // 128         # output chunks of h1 (8)
    nchunks = sl // SEG_CHUNK
    xlen = nt * feat2      # free elems per segment (3588)

    nc = bacc.Bacc(
        "TRN2", target_bir_lowering=False, debug=False, num_devices=n_cores
    )

    epk_d = nc.dram_tensor("epk", [128, sl * xlen], bf16, kind="ExternalInput").ap()
    wab_d = nc.dram_tensor("wab", [128, feat2], bf16, kind="ExternalInput").ap()
    oneh_d = nc.dram_tensor("oneh", [sl, ncom], f32, kind="ExternalInput").ap()
    w1_d = nc.dram_tensor("w1", [128, nj1 * h1], bf16, kind="ExternalInput").ap()
    w2_d = nc.dram_tensor("w2", [128, nj2 * h1], bf16, kind="ExternalInput").ap()
    w3_d = nc.dram_tensor("w3", [128, nj2 * ncls], bf16, kind="ExternalInput").ap()
    b1_d = nc.dram_tensor("b1", [1, h1], bf16, kind="ExternalInput").ap()
    b2_d = nc.dram_tensor("b2", [1, h1], bf16, kind="ExternalInput").ap()
    b3_d = nc.dram_tensor("b3", [1, ncls], bf16, kind="ExternalInput").ap()
    out_d = nc.dram_tensor("out", [ncom, ncls], f32, kind="ExternalOutput").ap()

    ar_in_d = nc.dram_tensor("ar_in", [128, nj1 * ncom], bf16).ap()
    ar_out_d = nc.dram_tensor(
        "ar_out", [128, nj1 * ncom], bf16
    ).ap()

    epk_v = epk_d.rearrange("p (s x) -> p s x", x=xlen)

    with tile.TileContext(nc) as tc:
        with (
            tc.tile_pool(name="const", bufs=1) as const_pool,
            tc.tile_pool(name="ep", bufs=3) as ep_pool,
            tc.tile_pool(name="work", bufs=1) as work,
            tc.tile_pool(name="psv", bufs=2, space="PSUM") as psv,
            tc.tile_pool(name="pmisc", bufs=2, space="PSUM") as pmisc,
        ):
            # ---- constants ----
            wab_sb = const_pool.tile([128, feat2], bf16)
            nc.sync.dma_start(wab_sb, wab_d)
            oneh_sb = const_pool.tile([sl, ncom], f32)
            nc.sync.dma_start(oneh_sb, oneh_d)
            ones_bf = const_pool.tile([128, ncom], bf16)
            nc.gpsimd.memset(ones_bf, 1.0)

            # ---- persistent working tiles ----
            L_sb = work.tile([128, nt * sl], f32)     # logits, col = s*nt + i
            E_sb = work.tile([128, nt * sl], bf16)    # exp(logits)
            prod = work.tile([128, feat2], bf16)      # STT product scratch
            segvecs = work.tile([sl, feat2 - pos], bf16)  # [s, 768 segvec | den]

            # ---- MLP weight tiles (bf16 straight from HBM) ----
            w1b = const_pool.tile([128, nj1, h1], bf16)
            w2b = const_pool.tile([128, nj2, h1], bf16)
            w3b = const_pool.tile([128, nj2, ncls], bf16)
            b1b = const_pool.tile([1, h1], bf16)
            b2b = const_pool.tile([1, h1], bf16)
            b3b = const_pool.tile([1, ncls], bf16)

            # One weight-DMA chunk or small-tensor load per input chunk;
            # interleaved into the segment loop so the HBM ring stays
            # dense and the loads fully overlap compute.
            w1f = w1b.rearrange("p j h -> p (j h)")
            w2f = w2b.rearrange("p j h -> p (j h)")
            dma_jobs = [
                lambda: nc.sync.dma_start(w1f[:, 0 : nj1 * h1 // 2], w1_d[:, 0 : nj1 * h1 // 2]),
                lambda: nc.sync.dma_start(w1f[:, nj1 * h1 // 2 :], w1_d[:, nj1 * h1 // 2 :]),
                lambda: nc.sync.dma_start(w2f[:, 0 : nj2 * h1 // 2], w2_d[:, 0 : nj2 * h1 // 2]),
                lambda: nc.sync.dma_start(w2f[:, nj2 * h1 // 2 :], w2_d[:, nj2 * h1 // 2 :]),
            ]

            def _small_loads():
                nc.sync.dma_start(w3b.rearrange("p j c -> p (j c)"), w3_d)
                nc.sync.dma_start(b1b, b1_d)
                nc.sync.dma_start(b2b, b2_d)
                nc.sync.dma_start(b3b, b3_d)

            dma_jobs.append(_small_loads)

            # ---- main loop over local segments, SEG_CHUNK at a time ----
            sv_tiles = {}

            def drain_segvec(sp):
                # PSUM -> SBUF stage (scalar, cast bf16), then scatter DMA.
                stage = work.tile([1, feat2 - pos], bf16, tag="stage", bufs=3, name="stage")
                nc.scalar.copy(stage, sv_tiles.pop(sp))
                nc.sync.dma_start(segvecs[sp : sp + 1, :], stage)

            for c in range(nchunks):
                ep = ep_pool.tile([128, SEG_CHUNK, nt, feat2], bf16, tag="ep")
                nc.sync.dma_start(
                    ep.rearrange("p s i f -> p s (i f)"),
                    epk_v[:, c * SEG_CHUNK : (c + 1) * SEG_CHUNK, :],
                )
                if c >= 1 and dma_jobs:
                    dma_jobs.pop(0)()

                for sc in range(SEG_CHUNK):
                    s = c * SEG_CHUNK + sc
                    # attention logits: fused multiply + free-dim reduce
                    for i in range(nt):
                        nc.vector.scalar_tensor_tensor(
                            prod,
                            ep[:, sc, i, :],
                            1.0,
                            wab_sb,
                            op0=OP.mult,
                            op1=OP.mult,
                            accum_out=L_sb[:, nt * s + i : nt * s + i + 1],
                        )
                    # e = exp(logits), bf16 out for the pooling stationary
                    nc.scalar.activation(
                        E_sb[:, nt * s : nt * s + nt],
                        L_sb[:, nt * s : nt * s + nt],
                        AF.Exp,
                    )
                    # Drain the PREVIOUS segment's pooled row here so the
                    # scalar stream goes exp(s) -> copy(s-1): copy(s-1)'s
                    # wait (on matmuls(s-1)) is already satisfied, so
                    # exp(s+1) never stalls behind matmuls(s).
                    if s >= 1:
                        drain_segvec(s - 1)

                    # pooling: segvec[s] = E-weighted sum over tokens; the
                    # ones column at 768 accumulates the denominator.
                    sv = psv.tile([1, feat2 - pos], f32, tag="sv")
                    sv_tiles[s] = sv
                    for i in range(nt):
                        col = nt * s + i
                        for n0, n1 in ((0, 512), (512, feat2 - pos)):
                            nc.tensor.matmul(
                                sv[0:1, n0:n1],
                                E_sb[:, col : col + 1],
                                ep[:, sc, i, n0:n1],
                                start=(i == 0),
                                stop=(i == nt - 1),
                            )

            drain_segvec(sl - 1)
            while dma_jobs:
                dma_jobs.pop(0)()

            # ---- denominators came along in the pooled rows ----
            inv_den = work.tile([sl, 1], f32)
            nc.vector.reciprocal(inv_den, segvecs[:, bert : bert + 1])
            oneh_sc = work.tile([sl, ncom], bf16)
            nc.vector.tensor_scalar_mul(oneh_sc, oneh_sb, inv_den)

            # ---- comment partials, transposed: cmT[f-chunk, j, c] ----
            cmT = pmisc.tile([128, nj1, ncom], f32, tag="m")
            for j in range(nj1):
                nc.tensor.matmul(
                    cmT[:, j, :],
                    segvecs[:, 128 * j : 128 * (j + 1)],
                    oneh_sc,
                    start=True,
                    stop=True,
                )
            arin_sb = work.tile([128, nj1 * ncom], bf16)
            nc.scalar.copy(arin_sb, cmT.rearrange("p j c -> p (j c)"))

            # ---- AllReduce over cores (bf16 payload, 98KB) ----
            nc.sync.dma_start(ar_in_d, arin_sb)
            nc.gpsimd.collective_compute(
                "AllReduce",
                OP.add,
                replica_groups=[list(range(n_cores))],
                ins=[ar_in_d],
                outs=[ar_out_d],
            )
            vecsT = work.tile([128, nj1, ncom], bf16)
            nc.sync.dma_start(vecsT.rearrange("p j c -> p (j c)"), ar_out_d)

            # ---- MLP, feature-major all the way (no transposes) ----
            def linearT(xT, njx, wb, brow, nchk):
                # hT[p, n, c] = sum_j wb[:, j, 128n+p]^T xT[:, j, c] + b
                hT = pmisc.tile([128, nchk, ncom], f32, tag="m", name="hT")
                for n in range(nchk):
                    for j in range(njx):
                        nc.tensor.matmul(
                            hT[:, n, :],
                            wb[:, j, 128 * n : 128 * (n + 1)],
                            xT[:, j, :],
                            start=(j == 0),
                            stop=False,
                        )
                    nc.tensor.matmul(
                        hT[:, n, :],
                        brow[0:1, 128 * n : 128 * (n + 1)],
                        ones_bf[0:1, 0:ncom],
                        start=False,
                        stop=True,
                    )
                # LeakyReLU on DVE: t = x*0.01 (PSUM->SBUF), y = max(t, x)
                hf = hT.rearrange("p n c -> p (n c)")
                tmp = work.tile([128, nchk * ncom], f32, tag="lr", bufs=2, name="lr")
                nc.vector.tensor_scalar_mul(tmp, hf, 0.01)
                yT = work.tile([128, nchk, ncom], bf16, tag="yT", bufs=2, name="yT")
                nc.vector.tensor_tensor(
                    yT.rearrange("p n c -> p (n c)"),
                    tmp,
                    hf,
                    op=OP.max,
                )
                return yT

            y1T = linearT(vecsT, nj1, w1b, b1b, nh)
            y2T = linearT(y1T, nj2, w2b, b2b, nh)

            # final layer directly as [com, cls]: y2 chunks stationary
            outP = pmisc.tile([ncom, ncls], f32, tag="m", name="outP")
            for j in range(nj2):
                nc.tensor.matmul(
                    outP,
                    y2T[:, j, :],
                    w3b[:, j, :],
                    start=(j == 0),
                    stop=False,
                )
            nc.tensor.matmul(
                outP,
                ones_bf[0:1, 0:ncom],
                b3b,
                start=False,
                stop=True,
            )
            # sigmoid(x) = 1 / (1 + exp(-x)) — Exp table already loaded
            en_sb = work.tile([ncom, ncls], f32)
            nc.scalar.activation(en_sb, outP, AF.Exp, scale=-1.0)
            p1_sb = work.tile([ncom, ncls], f32)
            nc.vector.tensor_scalar_add(p1_sb, en_sb, 1.0)
            out_sb = work.tile([ncom, ncls], f32)
            nc.vector.reciprocal(out_sb, p1_sb)
            nc.sync.dma_start(out_d, out_sb)

    nc.compile()
    return nc


def make_in_maps(
    embeddings,
    position_encodings,
    W_attn,
    W1,
    b1,
    W2,
    b2,
    W3,
    b3,
    segment_ids,
    n_cores,
    ncom,
):
    """Host-side sharding: pack bf16 partition-major inputs per core."""
    f32 = np.float32
    s_total = embeddings.shape[0]
    sl = s_total // n_cores
    bert = embeddings.shape[2]
    pos = position_encodings.shape[2]
    feat2 = bert + pos + 1
    nt = embeddings.shape[1] // 128
    nj1 = bert // 128
    h1 = W1.shape[1]
    nj2 = h1 // 128
    nh = h1 // 128
    ncls = W3.shape[1]

    # wab row: [W_e(768) | 0 | W_p(128)], tiled to 128 partitions
    wa = np.asarray(W_attn, dtype=f32).reshape(-1)
    row = np.concatenate([wa[:bert], [0.0], wa[bert:]]).astype(BF16)
    wab = np.ascontiguousarray(np.tile(row[None, :], (128, 1)))

    w1p = np.ascontiguousarray(
        np.asarray(W1, dtype=f32).astype(BF16).reshape(nj1, 128, h1).transpose(1, 0, 2)
    ).reshape(128, nj1 * h1)
    w2p = np.ascontiguousarray(
        np.asarray(W2, dtype=f32).astype(BF16).reshape(nj2, 128, h1).transpose(1, 0, 2)
    ).reshape(128, nj2 * h1)
    w3p = np.ascontiguousarray(
        np.asarray(W3, dtype=f32).astype(BF16).reshape(nj2, 128, ncls).transpose(1, 0, 2)
    ).reshape(128, nj2 * ncls)
    b1t = np.ascontiguousarray(np.asarray(b1, dtype=f32).reshape(1, -1).astype(BF16))
    b2t = np.ascontiguousarray(np.asarray(b2, dtype=f32).reshape(1, -1).astype(BF16))
    b3r = np.ascontiguousarray(np.asarray(b3, dtype=f32).reshape(1, -1).astype(BF16))

    seg = np.asarray(segment_ids).astype(np.int64).reshape(-1)
    common = {
        "wab": wab,
        "w1": w1p,
        "w2": w2p,
        "w3": w3p,
        "b1": b1t,
        "b2": b2t,
        "b3": b3r,
    }

    emb = np.asarray(embeddings, dtype=f32)
    posE = np.asarray(position_encodings, dtype=f32)

    in_maps = []
    for c in range(n_cores):
        oneh = np.zeros((sl, ncom), dtype=f32)
        local = seg[c * sl : (c + 1) * sl]
        oneh[np.arange(sl), local] = 1.0

        # pack [128, sl, nt, feat2] with cols [emb | 1.0 | pos]
        arr = np.empty((128, sl, nt, feat2), dtype=BF16)
        e_c = emb[c * sl : (c + 1) * sl].reshape(sl, nt, 128, bert)
        p_c = posE[c * sl : (c + 1) * sl].reshape(sl, nt, 128, pos)
        arr[:, :, :, 0:bert] = e_c.astype(BF16).transpose(2, 0, 1, 3)
        arr[:, :, :, bert] = BF16(1.0)
        arr[:, :, :, bert + 1 :] = p_c.astype(BF16).transpose(2, 0, 1, 3)
        epk = np.ascontiguousarray(arr.reshape(128, sl * nt * feat2))

        in_maps.append({"epk": epk, "oneh": oneh, **common})
    return in_maps


def kernel(
    embeddings,
    position_encodings,
    W_attn,
    b_attn,
    W1,
    b1,
    W2,
    b2,
    W3,
    b3,
    segment_ids,
    num_comments,
):
    from concourse.bass_utils import run_bass_kernel_spmd

    assert int(num_comments) == NCOM
    assert embeddings.shape == (S, T, BERT)
    assert position_encodings.shape == (S, T, POS)
    # b_attn shifts every logit of a segment equally -> softmax-invariant.

    key = "full"
    if key not in _CACHE:
        _CACHE[key] = build_nc(NCORES, S // NCORES, T, BERT, POS, H1, NCLS, NCOM)
    nc = _CACHE[key]

    in_maps = make_in_maps(
        embeddings,
        position_encodings,
        W_attn,
        W1,
        b1,
        W2,
        b2,
        W3,
        b3,
        segment_ids,
        NCORES,
        NCOM,
    )
    res = run_bass_kernel_spmd(nc, in_maps, list(range(NCORES)))
    return np.asarray(res.results[0]["out"], dtype=np.float32)


# revision 13
# speedup vs baseline: 1.7116x; 1.0726x over previous
"""Trainium2 Bass kernel for nn_Classifier (segment_reduce).

Computation (reference semantics):
  attn  = concat(emb, pos) @ W_attn + b_attn          (S, T, 1)
  w     = softmax(attn, axis=1)                        per-segment over T
  segv  = sum_t w * emb                                (S, BERT)
  vecs  = segment_sum(segv, segment_ids, 64)           (64, BERT)
  out   = sigmoid(lrelu(lrelu(vecs@W1+b1)@W2+b2)@W3+b3)

Sharding: data-parallel over S across 8 NeuronCores (32 segments each),
AllReduce of the comment partials (bf16), replicated MLP.

Structure (v2, bf16 end-to-end):
 - b_attn shifts all logits of a segment equally -> softmax-invariant -> dropped.
 - Host packs emb+pos per core into bf16, partition-major layout
   [128, sl, nt, 897] with columns [emb 768 | 1.0 | pos 128] so that
   (a) each 4-segment DMA is 128 x 28.7KB contiguous descriptors,
   (b) the pooling matmul over columns [512:769] accumulates the ones
       column -> softmax denominator lands in the pooled row for free.
   W_attn is zero-padded at the ones column so logits are unaffected.
 - Logits: DVE fused mul+reduce (bf16 in, fp32 accum), exp on scalar
   engine (bf16 out) -> pooling matmuls all-bf16 (4x PE stream rate).
 - Softmax 1/den folded into the host-built one-hot segment->comment
   matrix, scaled on device by reciprocal of the free den column.
 - Comment partials built transposed (cmT[feat, com]), AllReduced in
   bf16 (98KB) with a Shared-address output buffer (fast mesh path),
   and fed to a transpose-free feature-major MLP.
 - MLP: bias via k=1 matmuls, LeakyReLU as one DVE scalar_tensor_tensor
   (x*0.01 max x) per layer, final layer emitted directly as [64, 6]
   (y2 chunks stationary), sigmoid as exp(-x) -> 1/(1+e) on DVE so the
   scalar engine never swaps activation tables.
"""

import sys

sys.path.insert(0, "/opt/trn_rl_repo")

import ml_dtypes
import numpy as np

BF16 = ml_dtypes.bfloat16

# Full-problem dims (hardcoded per contract)
S, T, BERT, POS = 256, 512, 768, 128
FEAT = BERT + POS
FEAT2 = FEAT + 1  # [emb 768 | ones 1 | pos 128]
H1 = 1024
NCLS = 6
NCOM = 64
NCORES = 8
SEG_CHUNK = 4  # segments per input DMA

_CACHE = {}


def build_nc(n_cores, sl, t, bert, pos, h1, ncls, ncom):
    """Build the SPMD Bass program for one core (sl segments/core)."""
    import concourse.bass as bass
    import concourse.mybir as mybir
    import concourse.tile as tile
    from concourse import bacc

    f32 = mybir.dt.float32
    bf16 = mybir.dt.bfloat16
    AF = mybir.ActivationFunctionType
    OP = mybir.AluOpType

    feat2 = bert + pos + 1
    nt = t // 128          # token tiles per segment (4)
    nj1 = bert // 128      # k tiles layer1 (6)
    nj2 = h1 // 128        # k tiles layer2/3 (8)
    nh = h1 // 128         # output chunks of h1 (8)
    nchunks = sl // SEG_CHUNK
    xlen = nt * feat2      # free elems per segment (3588)

    nc = bacc.Bacc(
        "TRN2", target_bir_lowering=False, debug=False, num_devices=n_cores
    )

    epk_d = nc.dram_tensor("epk", [128, sl * xlen], bf16, kind="ExternalInput").ap()
    wab_d = nc.dram_tensor("wab", [128, feat2], bf16, kind="ExternalInput").ap()
    oneh_d = nc.dram_tensor("oneh", [sl, ncom], f32, kind="ExternalInput").ap()
    w1_d = nc.dram_tensor("w1", [128, nj1 * h1], bf16, kind="ExternalInput").ap()
    w2_d = nc.dram_tensor("w2", [128, nj2 * h1], bf16, kind="ExternalInput").ap()
    w3_d = nc.dram_tensor("w3", [128, nj2 * ncls], bf16, kind="ExternalInput").ap()
    b1_d = nc.dram_tensor("b1", [1, h1], bf16, kind="ExternalInput").ap()
    b2_d = nc.dram_tensor("b2", [1, h1], bf16, kind="ExternalInput").ap()
    b3_d = nc.dram_tensor("b3", [1, ncls], bf16, kind="ExternalInput").ap()
    out_d = nc.dram_tensor("out", [ncom, ncls], f32, kind="ExternalOutput").ap()

    ag_in_d = nc.dram_tensor("ag_in", [128, nj1 * ncom], bf16).ap()
    ag_out_d = nc.dram_tensor("ag_out", [n_cores * 128, nj1 * ncom], bf16).ap()

    epk_v = epk_d.rearrange("p (s x) -> p s x", x=xlen)

    with tile.TileContext(nc) as tc:
        with (
            tc.tile_pool(name="const", bufs=1) as const_pool,
            tc.tile_pool(name="ep", bufs=4) as ep_pool,
            tc.tile_pool(name="work", bufs=1) as work,
            tc.tile_pool(name="psv", bufs=2, space="PSUM") as psv,
            tc.tile_pool(name="pmisc", bufs=2, space="PSUM") as pmisc,
        ):
            # ---- constants ----
            wab_sb = const_pool.tile([128, feat2], bf16)
            nc.sync.dma_start(wab_sb, wab_d)
            oneh_sb = const_pool.tile([sl, ncom], f32)
            nc.sync.dma_start(oneh_sb, oneh_d)
            ones_bf = const_pool.tile([128, ncom], bf16)
            nc.gpsimd.memset(ones_bf, 1.0)

            # ---- persistent working tiles ----
            L_sb = work.tile([128, nt * sl], f32)     # logits, col = s*nt + i
            E_sb = work.tile([128, nt * sl], bf16)    # exp(logits)
            prod = work.tile([128, feat2], bf16)      # STT product scratch
            dump = work.tile([128, feat2], bf16)      # ACT accum-path out sink
            segvecs = work.tile([sl, feat2 - pos], bf16)  # [s, 768 segvec | den]

            # ---- MLP weight tiles (bf16 straight from HBM) ----
            w1b = const_pool.tile([128, nj1, h1], bf16)
            w2b = const_pool.tile([128, nj2, h1], bf16)
            w3b = const_pool.tile([128, nj2, ncls], bf16)
            b1b = const_pool.tile([1, h1], bf16)
            b2b = const_pool.tile([1, h1], bf16)
            b3b = const_pool.tile([1, ncls], bf16)

            # One weight-DMA chunk or small-tensor load per input chunk;
            # interleaved into the segment loop so the HBM ring stays
            # dense and the loads fully overlap compute.
            w1f = w1b.rearrange("p j h -> p (j h)")
            w2f = w2b.rearrange("p j h -> p (j h)")
            dma_jobs = [
                lambda: nc.sync.dma_start(w1f[:, 0 : nj1 * h1 // 2], w1_d[:, 0 : nj1 * h1 // 2]),
                lambda: nc.sync.dma_start(w1f[:, nj1 * h1 // 2 :], w1_d[:, nj1 * h1 // 2 :]),
                lambda: nc.sync.dma_start(w2f[:, 0 : nj2 * h1 // 2], w2_d[:, 0 : nj2 * h1 // 2]),
                lambda: nc.sync.dma_start(w2f[:, nj2 * h1 // 2 :], w2_d[:, nj2 * h1 // 2 :]),
            ]

            def _small_loads():
                nc.sync.dma_start(w3b.rearrange("p j c -> p (j c)"), w3_d)
                nc.sync.dma_start(b1b, b1_d)
                nc.sync.dma_start(b2b, b2_d)
                nc.sync.dma_start(b3b, b3_d)

            dma_jobs.append(_small_loads)

            # ---- main loop over local segments, SEG_CHUNK at a time ----
            sv_tiles = {}

            def drain_segvec(sp):
                # PSUM -> SBUF stage (scalar, cast bf16), then scatter DMA.
                stage = work.tile([1, feat2 - pos], bf16, tag="stage", bufs=3, name="stage")
                nc.scalar.copy(stage, sv_tiles.pop(sp))
                nc.sync.dma_start(segvecs[sp : sp + 1, :], stage)

            for c in range(nchunks):
                ep = ep_pool.tile([128, SEG_CHUNK, nt, feat2], bf16, tag="ep")
                nc.sync.dma_start(
                    ep.rearrange("p s i f -> p s (i f)"),
                    epk_v[:, c * SEG_CHUNK : (c + 1) * SEG_CHUNK, :],
                )
                if c >= 1 and dma_jobs:
                    dma_jobs.pop(0)()

                for sc in range(SEG_CHUNK):
                    s = c * SEG_CHUNK + sc
                    # Attention logits, split across DVE and ACT so neither
                    # engine paces the loop alone:
                    #  - tiles 2,3: DVE tensor_tensor product (bf16, 2x mode)
                    #    then ACT Copy+accum_out does the free-dim reduce
                    #  - tiles 0,1: DVE fused STT (1x, but single op)
                    # DVE emits the TT products first so ACT starts early.
                    p2 = {}
                    for i in (2, 3):
                        p2[i] = work.tile(
                            [128, feat2], bf16, tag="p2", bufs=4, name="p2"
                        )
                        nc.vector.tensor_tensor(
                            p2[i], ep[:, sc, i, :], wab_sb, op=OP.mult
                        )
                    for i in (0, 1):
                        nc.vector.scalar_tensor_tensor(
                            prod,
                            ep[:, sc, i, :],
                            1.0,
                            wab_sb,
                            op0=OP.mult,
                            op1=OP.mult,
                            accum_out=L_sb[:, nt * s + i : nt * s + i + 1],
                        )
                    for i in (2, 3):
                        nc.scalar.activation(
                            dump,
                            p2[i],
                            AF.Copy,
                            accum_out=L_sb[:, nt * s + i : nt * s + i + 1],
                        )
                    # e = exp(logits), bf16 out for the pooling stationary
                    nc.scalar.activation(
                        E_sb[:, nt * s : nt * s + nt],
                        L_sb[:, nt * s : nt * s + nt],
                        AF.Exp,
                    )
                    # Drain the PREVIOUS segment's pooled row here so the
                    # scalar stream goes exp(s) -> copy(s-1): copy(s-1)'s
                    # wait (on matmuls(s-1)) is already satisfied, so
                    # exp(s+1) never stalls behind matmuls(s).
                    if s >= 1:
                        drain_segvec(s - 1)

                    # pooling: segvec[s] = E-weighted sum over tokens; the
                    # ones column at 768 accumulates the denominator.
                    sv = psv.tile([1, feat2 - pos], f32, tag="sv")
                    sv_tiles[s] = sv
                    for i in range(nt):
                        col = nt * s + i
                        for n0, n1 in ((0, 512), (512, feat2 - pos)):
                            nc.tensor.matmul(
                                sv[0:1, n0:n1],
                                E_sb[:, col : col + 1],
                                ep[:, sc, i, n0:n1],
                                start=(i == 0),
                                stop=(i == nt - 1),
                            )

            drain_segvec(sl - 1)
            while dma_jobs:
                dma_jobs.pop(0)()

            # ---- denominators came along in the pooled rows ----
            inv_den = work.tile([sl, 1], f32)
            nc.vector.reciprocal(inv_den, segvecs[:, bert : bert + 1])
            oneh_sc = work.tile([sl, ncom], bf16)
            nc.vector.tensor_scalar_mul(oneh_sc, oneh_sb, inv_den)

            # ---- comment partials, transposed: cmT[f-chunk, j, c] ----
            cmT = pmisc.tile([128, nj1, ncom], f32, tag="m")
            for j in range(nj1):
                nc.tensor.matmul(
                    cmT[:, j, :],
                    segvecs[:, 128 * j : 128 * (j + 1)],
                    oneh_sc,
                    start=True,
                    stop=True,
                )
            arin_sb = work.tile([128, nj1 * ncom], bf16)
            nc.scalar.copy(arin_sb, cmT.rearrange("p j c -> p (j c)"))

            # ---- AllGather partials (low collective floor), reduce on DVE ----
            nc.sync.dma_start(ag_in_d, arin_sb)
            nc.gpsimd.collective_compute(
                "AllGather",
                OP.bypass,
                replica_groups=[list(range(n_cores))],
                ins=[ag_in_d],
                outs=[ag_out_d],
            )
            parts = work.tile([128, n_cores, nj1 * ncom], bf16)
            nc.sync.dma_start(
                parts, ag_out_d.rearrange("(r p) x -> p r x", p=128)
            )
            # binary tree sum of the 8 rank partials (bf16 TT adds, 2x mode)
            t4 = work.tile([128, 4, nj1 * ncom], bf16)
            for k in range(4):
                nc.vector.tensor_tensor(
                    t4[:, k, :], parts[:, 2 * k, :], parts[:, 2 * k + 1, :], op=OP.add
                )
            t2 = work.tile([128, 2, nj1 * ncom], bf16)
            for k in range(2):
                nc.vector.tensor_tensor(
                    t2[:, k, :], t4[:, 2 * k, :], t4[:, 2 * k + 1, :], op=OP.add
                )
            vecsT = work.tile([128, nj1, ncom], bf16)
            nc.vector.tensor_tensor(
                vecsT.rearrange("p j c -> p (j c)"), t2[:, 0, :], t2[:, 1, :], op=OP.add
            )

            # ---- MLP, feature-major all the way (no transposes) ----
            def linearT(xT, njx, wb, brow, nchk):
                # hT[p, n, c] = sum_j wb[:, j, 128n+p]^T xT[:, j, c] + b
                hT = pmisc.tile([128, nchk, ncom], f32, tag="m", name="hT")
                for n in range(nchk):
                    for j in range(njx):
                        nc.tensor.matmul(
                            hT[:, n, :],
                            wb[:, j, 128 * n : 128 * (n + 1)],
                            xT[:, j, :],
                            start=(j == 0),
                            stop=False,
                        )
                    nc.tensor.matmul(
                        hT[:, n, :],
                        brow[0:1, 128 * n : 128 * (n + 1)],
                        ones_bf[0:1, 0:ncom],
                        start=False,
                        stop=True,
                    )
                # LeakyReLU on DVE: t = x*0.01 (PSUM->SBUF), y = max(t, x)
                hf = hT.rearrange("p n c -> p (n c)")
                tmp = work.tile([128, nchk * ncom], f32, tag="lr", bufs=2, name="lr")
                nc.vector.tensor_scalar_mul(tmp, hf, 0.01)
                yT = work.tile([128, nchk, ncom], bf16, tag="yT", bufs=2, name="yT")
                nc.vector.tensor_tensor(
                    yT.rearrange("p n c -> p (n c)"),
                    tmp,
                    hf,
                    op=OP.max,
                )
                return yT

            y1T = linearT(vecsT, nj1, w1b, b1b, nh)
            y2T = linearT(y1T, nj2, w2b, b2b, nh)

            # final layer directly as [com, cls]: y2 chunks stationary
            outP = pmisc.tile([ncom, ncls], f32, tag="m", name="outP")
            for j in range(nj2):
                nc.tensor.matmul(
                    outP,
                    y2T[:, j, :],
                    w3b[:, j, :],
                    start=(j == 0),
                    stop=False,
                )
            nc.tensor.matmul(
                outP,
                ones_bf[0:1, 0:ncom],
                b3b,
                start=False,
                stop=True,
            )
            # sigmoid(x) = 1 / (1 + exp(-x)) — Exp table already loaded
            en_sb = work.tile([ncom, ncls], f32)
            nc.scalar.activation(en_sb, outP, AF.Exp, scale=-1.0)
            p1_sb = work.tile([ncom, ncls], f32)
            nc.vector.tensor_scalar_add(p1_sb, en_sb, 1.0)
            out_sb = work.tile([ncom, ncls], f32)
            nc.vector.reciprocal(out_sb, p1_sb)
            nc.sync.dma_start(out_d, out_sb)

    nc.compile()
    return nc


def make_in_maps(
    embeddings,
    position_encodings,
    W_attn,
    W1,
    b1,
    W2,
    b2,
    W3,
    b3,
    segment_ids,
    n_cores,
    ncom,
):
    """Host-side sharding: pack bf16 partition-major inputs per core."""
    f32 = np.float32
    s_total = embeddings.shape[0]
    sl = s_total // n_cores
    bert = embeddings.shape[2]
    pos = position_encodings.shape[2]
    feat2 = bert + pos + 1
    nt = embeddings.shape[1] // 128
    nj1 = bert // 128
    h1 = W1.shape[1]
    nj2 = h1 // 128
    nh = h1 // 128
    ncls = W3.shape[1]

    # wab row: [W_e(768) | 0 | W_p(128)], tiled to 128 partitions
    wa = np.asarray(W_attn, dtype=f32).reshape(-1)
    row = np.concatenate([wa[:bert], [0.0], wa[bert:]]).astype(BF16)
    wab = np.ascontiguousarray(np.tile(row[None, :], (128, 1)))

    w1p = np.ascontiguousarray(
        np.asarray(W1, dtype=f32).astype(BF16).reshape(nj1, 128, h1).transpose(1, 0, 2)
    ).reshape(128, nj1 * h1)
    w2p = np.ascontiguousarray(
        np.asarray(W2, dtype=f32).astype(BF16).reshape(nj2, 128, h1).transpose(1, 0, 2)
    ).reshape(128, nj2 * h1)
    w3p = np.ascontiguousarray(
        np.asarray(W3, dtype=f32).astype(BF16).reshape(nj2, 128, ncls).transpose(1, 0, 2)
    ).reshape(128, nj2 * ncls)
    b1t = np.ascontiguousarray(np.asarray(b1, dtype=f32).reshape(1, -1).astype(BF16))
    b2t = np.ascontiguousarray(np.asarray(b2, dtype=f32).reshape(1, -1).astype(BF16))
    b3r = np.ascontiguousarray(np.asarray(b3, dtype=f32).reshape(1, -1).astype(BF16))

    seg = np.asarray(segment_ids).astype(np.int64).reshape(-1)
    common = {
        "wab": wab,
        "w1": w1p,
        "w2": w2p,
        "w3": w3p,
        "b1": b1t,
        "b2": b2t,
        "b3": b3r,
    }

    emb = np.asarray(embeddings, dtype=f32)
    posE = np.asarray(position_encodings, dtype=f32)

    in_maps = []
    for c in range(n_cores):
        oneh = np.zeros((sl, ncom), dtype=f32)
        local = seg[c * sl : (c + 1) * sl]
        oneh[np.arange(sl), local] = 1.0

        # pack [128, sl, nt, feat2] with cols [emb | 1.0 | pos]
        arr = np.empty((128, sl, nt, feat2), dtype=BF16)
        e_c = emb[c * sl : (c + 1) * sl].reshape(sl, nt, 128, bert)
        p_c = posE[c * sl : (c + 1) * sl].reshape(sl, nt, 128, pos)
        arr[:, :, :, 0:bert] = e_c.astype(BF16).transpose(2, 0, 1, 3)
        arr[:, :, :, bert] = BF16(1.0)
        arr[:, :, :, bert + 1 :] = p_c.astype(BF16).transpose(2, 0, 1, 3)
        epk = np.ascontiguousarray(arr.reshape(128, sl * nt * feat2))

        in_maps.append({"epk": epk, "oneh": oneh, **common})
    return in_maps


def kernel(
    embeddings,
    position_encodings,
    W_attn,
    b_attn,
    W1,
    b1,
    W2,
    b2,
    W3,
    b3,
    segment_ids,
    num_comments,
):
    from concourse.bass_utils import run_bass_kernel_spmd

    assert int(num_comments) == NCOM
    assert embeddings.shape == (S, T, BERT)
    assert position_encodings.shape == (S, T, POS)
    # b_attn shifts every logit of a segment equally -> softmax-invariant.

    key = "full"
    if key not in _CACHE:
        _CACHE[key] = build_nc(NCORES, S // NCORES, T, BERT, POS, H1, NCLS, NCOM)
    nc = _CACHE[key]

    in_maps = make_in_maps(
        embeddings,
        position_encodings,
        W_attn,
        W1,
        b1,
        W2,
        b2,
        W3,
        b3,
        segment_ids,
        NCORES,
        NCOM,
    )
    res = run_bass_kernel_spmd(nc, in_maps, list(range(NCORES)))
    return np.asarray(res.results[0]["out"], dtype=np.float32)
